# revision 41
# baseline (speedup 1.0000x reference)
"""v4: (16,32)-factorized one-hot histogram, balanced across DVE+Pool+Act.

Binning: per-coord magic-number rounds on the Act engine produce pre-scaled
integer coords directly:
    q0t = 4*rne(2*x0+4.5)   via z=8*x0+18,  +-2^25 (ulp 4)
    q1t = 16*rne(2*x1+4.5)  via z=32*x1+72, +-2^27 (ulp 16)
    q2t =    rne(2*x2+4.5)  via z=2*x2+4.5, +-2^23 (ulp 1)
    s   = Sign(x1)
Factor values (all bf16-exact, built with 2x/4x-mode DVE ops):
    g = q0t + s            -> 16 odd targets {3,5,...,33}
    h = (q1t + q2t) - 32*s -> 32 targets {16k+r: k 3..6, r 1..8}
Invalid coords (outside [-2,2]) produce values outside the target sets
(coarser/finer rounding granules below the magic threshold give even/half-
integer values that never collide) and drop out free.

Engine split per group: Act runs the 9 affine/round ops + Sign + per-batch
PSUM->SBUF cnt copies; DVE runs 3 TT-adds + 1 TS and the G one-hot + first
32-H_POOL rows of H (tensor_tensor is_equal, bf16 2x); Pool builds the last
H_POOL rows via per-row tensor_scalar is_equal. PE accumulates ps[32,16] per
batch (64 matmuls, 16 cycles each), stage-2 contracts h with hi/lo bf16 W,
split at the half-way point to shorten the tail. Group sizes ramp 2,2,4 /
8... / 4,2,2 to shrink pipeline fill/drain.
"""

import numpy as np

B, N, VR, CLS = 1024, 8192, 8, 40
NCORES = 8
BPC = B // NCORES
PJ = N // 128
M23 = 8388608.0    # 2^23
M25 = 33554432.0   # 2^25
M27 = 134217728.0  # 2^27
GRP = 8            # max batches per instruction group
G_POOL = 10        # G one-hot rows built on Pool (g is ready earliest)

_CACHE = {}

_GI = np.arange(16)
_G_VALS = 4 * (_GI // 2 + 1) + 2 * (_GI % 2) - 1      # odd ints 3..33
_HI = np.arange(32)
_H_VALS = 16 * (_HI // 8 + 3) + _HI % 8 + 1           # ints 49..104
_CONSTS = (18.0, 72.0, 4.5, M25, -M25, M27, -M27, M23, -M23,
           -102.0, -103.0, -104.0, 1.0)


def _build(n_batches):
    import concourse.bacc as bacc
    import concourse.mybir as mybir
    import concourse.tile as tile

    dt = mybir.dt
    op = mybir.AluOpType
    AF = mybir.ActivationFunctionType
    nc = bacc.Bacc("TRN2", target_bir_lowering=False, debug=False,
                   num_devices=NCORES)

    x_d = nc.dram_tensor("x", (128, n_batches, 3, PJ), dt.float32,
                         kind="ExternalInput")
    w2_d = nc.dram_tensor("w2", (32, 2, 16, CLS), dt.bfloat16,
                          kind="ExternalInput")
    bias_d = nc.dram_tensor("bias", (CLS, 1), dt.float32,
                            kind="ExternalInput")
    cst_d = nc.dram_tensor("cst", (1, len(_CONSTS)), dt.float32,
                           kind="ExternalInput")
    y_d = nc.dram_tensor("y", (CLS, n_batches), dt.float32,
                         kind="ExternalOutput")

    assert n_batches == 128
    GROUPS = globals().get("_GROUPS_OVERRIDE") or [4, 4] + [8] * 15
    S2_CUTS = [64, 96, 120, 128]
    GD = 16 - G_POOL   # G rows on DVE

    with tile.TileContext(nc) as tc:
        with (
            tc.tile_pool(name="const", bufs=1) as cpool,
            tc.tile_pool(name="x", bufs=3) as xpool,
            tc.tile_pool(name="q", bufs=3) as qpool,
            tc.tile_pool(name="hq", bufs=1) as hqpool,
            tc.tile_pool(name="v", bufs=3) as vpool,
            tc.tile_pool(name="oh", bufs=globals().get("_OHBUFS") or 2) as ohpool,
            tc.tile_pool(name="cnt", bufs=1) as cntpool,
            tc.tile_pool(name="ps1", bufs=3, space="PSUM") as ps1pool,
            tc.tile_pool(name="ps2", bufs=1, space="PSUM") as ps2pool,
        ):
            consts = cpool.tile([128, len(_CONSTS)], dt.float32)
            nc.sync.dma_start(
                consts[:], cst_d.ap().broadcast_to((128, len(_CONSTS))))
            cb = {v: consts[:, i:i + 1] for i, v in enumerate(_CONSTS)}

            # prefetch the first groups' x before the big const tiles so the
            # Act chain starts as early as possible
            pre_n = 3
            pre_xg = []
            pb = 0
            for gs in GROUPS[:pre_n]:
                xg = xpool.tile([128, GRP, 3, PJ], dt.float32,
                                tag="xg", name="xg")[:, 0:gs]
                nc.sync.dma_start(xg[:], x_d[:, pb:pb + gs])
                pre_xg.append(xg)
                pb += gs

            w2 = cpool.tile([32, 2, 16, CLS], dt.bfloat16)
            nc.sync.dma_start(w2[:], w2_d[:])
            bias = cpool.tile([CLS, 1], dt.float32)
            nc.sync.dma_start(bias[:], bias_d[:])

            cnt = cntpool.tile([32, n_batches, 16], dt.bfloat16)
            ps2 = ps2pool.tile([CLS, n_batches], dt.float32)

            b0 = 0
            pending = []   # deferred (ps1, pos) cnt copies from prev group
            copied = [0]
            s2_done = [0]

            def flush_pending():
                for ps1t, pos, nb_ in pending:
                    nc.scalar.copy(cnt[:, pos:pos + nb_, :],
                                   ps1t[:, 0:nb_])
                    copied[0] = pos + nb_
                pending.clear()
                for cut in S2_CUTS:
                    if s2_done[0] < cut <= copied[0]:
                        c0 = s2_done[0]
                        for hl in range(2):
                            for gg in range(16):
                                nc.tensor.matmul(ps2[:, c0:cut],
                                                 w2[:, hl, gg],
                                                 cnt[:, c0:cut, gg],
                                                 start=(hl == 0 and gg == 0),
                                                 stop=(hl == 1 and gg == 15))
                        out_t = out[0]
                        if out_t is None:
                            out_t = cpool.tile([CLS, n_batches], dt.float32,
                                               tag="out", name="out")
                            out[0] = out_t
                        nc.vector.tensor_scalar(out_t[:, c0:cut],
                                                ps2[:, c0:cut], 1.0 / N,
                                                bias[:], op.mult, op.add)
                        nc.sync.dma_start(y_d[:, c0:cut], out_t[:, c0:cut])
                        s2_done[0] = cut

            out = [None]
            for gi_, gs in enumerate(GROUPS):
                if gi_ < pre_n:
                    xg = pre_xg[gi_]
                else:
                    xg = xpool.tile([128, GRP, 3, PJ], dt.float32,
                                    tag="xg", name="xg")[:, 0:gs]
                    nc.sync.dma_start(xg[:], x_d[:, b0:b0 + gs])

                s = vpool.tile([128, GRP, PJ], dt.bfloat16, tag="s", name="s")[:, 0:gs]
                nc.scalar.activation(s[:], xg[:, :, 1], AF.Sign)
                s32 = vpool.tile([128, GRP, PJ], dt.bfloat16,
                                 tag="s32", name="s32")[:, 0:gs]
                nc.scalar.mul(s32[:], s[:], -32.0)

                g = vpool.tile([128, GRP, PJ], dt.bfloat16, tag="g", name="g")[:, 0:gs]
                lin = vpool.tile([128, GRP, PJ], dt.bfloat16,
                                 tag="lin", name="lin")[:, 0:gs]
                h = vpool.tile([128, GRP, PJ], dt.bfloat16, tag="h", name="h")[:, 0:gs]
                if gi_ < 2:
                    # head groups: short 3-op joint Act chain + 1x STT
                    # combines so DVE starts as early as possible
                    tj = hqpool.tile([128, 4, 3, PJ], dt.float32,
                                     tag="tj", name="tj")[:, 0:gs]
                    nc.scalar.activation(tj[:], xg[:], AF.Identity,
                                         bias=cb[4.5], scale=2.0)
                    qaj = hqpool.tile([128, 4, 3, PJ], dt.float32,
                                      tag="qaj", name="qaj")[:, 0:gs]
                    nc.scalar.activation(qaj[:], tj[:], AF.Identity,
                                         bias=cb[M23])
                    qj = hqpool.tile([128, 4, 3, PJ], dt.bfloat16,
                                     tag="qj", name="qj")[:, 0:gs]
                    nc.scalar.activation(qj[:], qaj[:], AF.Identity,
                                         bias=cb[-M23])
                    flush_pending()
                    nc.vector.scalar_tensor_tensor(g[:], qj[:, :, 0], 4.0,
                                                   s[:], op.mult, op.add)
                    nc.vector.scalar_tensor_tensor(lin[:], qj[:, :, 1], 16.0,
                                                   qj[:, :, 2], op.mult,
                                                   op.add)
                    nc.vector.scalar_tensor_tensor(h[:], s[:], -32.0, lin[:],
                                                   op.mult, op.add)
                else:
                    qt = []
                    for ci, (scale, coff, cp, cn) in enumerate([
                            (8.0, 18.0, M25, -M25),
                            (32.0, 72.0, M27, -M27),
                            (2.0, 4.5, M23, -M23)]):
                        t = qpool.tile([128, GRP, PJ], dt.float32,
                                       tag=f"t{ci}", name=f"t{ci}")[:, 0:gs]
                        nc.scalar.activation(t[:], xg[:, :, ci], AF.Identity,
                                             bias=cb[coff], scale=scale)
                        qa = qpool.tile([128, GRP, PJ], dt.float32,
                                        tag=f"qa{ci}", name=f"qa{ci}")[:, 0:gs]
                        nc.scalar.activation(qa[:], t[:], AF.Identity,
                                             bias=cb[cp])
                        qq = qpool.tile([128, GRP, PJ], dt.bfloat16,
                                        tag=f"q{ci}", name=f"q{ci}")[:, 0:gs]
                        nc.scalar.activation(qq[:], qa[:], AF.Identity,
                                             bias=cb[cn])
                        qt.append(qq)

                    flush_pending()

                    nc.vector.tensor_tensor(g[:], qt[0][:], s[:], op.add)
                    nc.vector.tensor_tensor(lin[:], qt[1][:], qt[2][:],
                                            op.add)
                    nc.vector.tensor_tensor(h[:], lin[:], s32[:], op.add)

                G = ohpool.tile([128, GRP, 16, PJ], dt.bfloat16,
                                tag="G", name="G")[:, 0:gs]
                H = ohpool.tile([128, GRP, 32, PJ], dt.bfloat16,
                                tag="H", name="H")[:, 0:gs]
                gd_ = GD - (gi_ % 2)   # alternate 10/11 Pool G-rows
                halves = ([(0, gs)] if gi_ < len(GROUPS) - 1 else
                          [(0, gs // 2), (gs // 2, gs)])
                for lo, hi in halves:
                    for m in range(gd_, 16):
                        nc.gpsimd.tensor_scalar(G[:, lo:hi, m], g[:, lo:hi],
                                                float(_G_VALS[m]), None,
                                                op.is_equal)
                    for m in range(gd_):
                        nc.vector.tensor_scalar(G[:, lo:hi, m], g[:, lo:hi],
                                                float(_G_VALS[m]), None,
                                                op.is_equal)
                    for m in range(32):
                        nc.vector.tensor_scalar(H[:, lo:hi, m], h[:, lo:hi],
                                                float(_H_VALS[m]), None,
                                                op.is_equal)

                    for bb in range(lo, hi, 8):
                        nb_ = min(8, hi - bb)
                        ps1 = ps1pool.tile([32, 8, 16], dt.float32, tag="ps1",
                                           name="ps1")
                        for u in range(nb_):
                            for j in range(PJ):
                                nc.tensor.matmul(ps1[:, u],
                                                 H[:, bb + u, :, j],
                                                 G[:, bb + u, :, j],
                                                 start=(j == 0),
                                                 stop=(j == PJ - 1))
                        pending.append((ps1, b0 + bb, nb_))

                b0 += gs
                if gi_ == len(GROUPS) - 1:
                    flush_pending()


            flush_pending()

    nc.compile()
    return nc


def _aux_inputs(W, b):
    from ml_dtypes import bfloat16 as bf16
    i0 = _GI // 2                                   # [16]
    i1 = (_HI // 8)[:, None] + 4 * (_GI % 2)[None, :]   # [32, 16]
    i2 = _HI % 8                                    # [32]
    vox = 64 * i0[None, :] + 8 * i1 + i2[:, None]   # [32, 16]
    w2f = np.ascontiguousarray(
        W[:, vox].transpose(1, 2, 0)).astype(np.float32)   # [32, 16, 40]
    w2hi = w2f.astype(bf16)
    w2lo = (w2f - w2hi.astype(np.float32)).astype(bf16)
    w2 = np.ascontiguousarray(np.stack([w2hi, w2lo], axis=1))  # [32,2,16,40]
    bias = np.asarray(b, dtype=np.float32).reshape(CLS, 1)
    cst = np.asarray(_CONSTS, dtype=np.float32).reshape(1, len(_CONSTS))
    return w2, bias, cst


def kernel(x, W, b):
    from concourse.bass_utils import run_bass_kernel_spmd

    x = np.asarray(x, dtype=np.float32)
    W = np.asarray(W, dtype=np.float32)
    b = np.asarray(b, dtype=np.float32)

    if BPC not in _CACHE:
        _CACHE[BPC] = _build(BPC)
    nc = _CACHE[BPC]

    w2, bias, cst = _aux_inputs(W, b)
    # [core, 128part, nb, 3coord, PJ]
    shards = x.reshape(NCORES, BPC, 128, PJ, 3).transpose(0, 2, 1, 4, 3)
    in_maps = [
        {"x": np.ascontiguousarray(shards[i]), "w2": w2,
         "bias": bias, "cst": cst}
        for i in range(NCORES)
    ]
    res = run_bass_kernel_spmd(nc, in_maps, list(range(NCORES)))
    return np.concatenate(
        [np.asarray(res.results[i]["y"]).T for i in range(NCORES)],
        axis=0).astype(np.float32)


# revision 42
# speedup vs baseline: 1.0092x; 1.0092x over previous
"""v4: (16,32)-factorized one-hot histogram, balanced across DVE+Pool+Act.

Binning: per-coord magic-number rounds on the Act engine produce pre-scaled
integer coords directly:
    q0t = 4*rne(2*x0+4.5)   via z=8*x0+18,  +-2^25 (ulp 4)
    q1t = 16*rne(2*x1+4.5)  via z=32*x1+72, +-2^27 (ulp 16)
    q2t =    rne(2*x2+4.5)  via z=2*x2+4.5, +-2^23 (ulp 1)
    s   = Sign(x1)
Factor values (all bf16-exact, built with 2x/4x-mode DVE ops):
    g = q0t + s            -> 16 odd targets {3,5,...,33}
    h = (q1t + q2t) - 32*s -> 32 targets {16k+r: k 3..6, r 1..8}
Invalid coords (outside [-2,2]) produce values outside the target sets
(coarser/finer rounding granules below the magic threshold give even/half-
integer values that never collide) and drop out free.

Engine split per group: Act runs the 9 affine/round ops + Sign + per-batch
PSUM->SBUF cnt copies; DVE runs 3 TT-adds + 1 TS and the G one-hot + first
32-H_POOL rows of H (tensor_tensor is_equal, bf16 2x); Pool builds the last
H_POOL rows via per-row tensor_scalar is_equal. PE accumulates ps[32,16] per
batch (64 matmuls, 16 cycles each), stage-2 contracts h with hi/lo bf16 W,
split at the half-way point to shorten the tail. Group sizes ramp 2,2,4 /
8... / 4,2,2 to shrink pipeline fill/drain.
"""

import numpy as np

B, N, VR, CLS = 1024, 8192, 8, 40
NCORES = 8
BPC = B // NCORES
PJ = N // 128
M23 = 8388608.0    # 2^23
M25 = 33554432.0   # 2^25
M27 = 134217728.0  # 2^27
GRP = 8            # max batches per instruction group
G_POOL = 10        # G one-hot rows built on Pool (g is ready earliest)

_CACHE = {}

_GI = np.arange(16)
_G_VALS = 4 * (_GI // 2 + 1) + 2 * (_GI % 2) - 1      # odd ints 3..33
_HI = np.arange(32)
_H_VALS = 16 * (_HI // 8 + 3) + _HI % 8 + 1           # ints 49..104
_CONSTS = (18.0, 72.0, 4.5, M25, -M25, M27, -M27, M23, -M23,
           -102.0, -103.0, -104.0, 1.0)


def _build(n_batches):
    import concourse.bacc as bacc
    import concourse.mybir as mybir
    import concourse.tile as tile

    dt = mybir.dt
    op = mybir.AluOpType
    AF = mybir.ActivationFunctionType
    nc = bacc.Bacc("TRN2", target_bir_lowering=False, debug=False,
                   num_devices=NCORES)

    x_d = nc.dram_tensor("x", (128, n_batches, 3, PJ), dt.float32,
                         kind="ExternalInput")
    w2_d = nc.dram_tensor("w2", (32, 2, 16, CLS), dt.bfloat16,
                          kind="ExternalInput")
    bias_d = nc.dram_tensor("bias", (CLS, 1), dt.float32,
                            kind="ExternalInput")
    cst_d = nc.dram_tensor("cst", (1, len(_CONSTS)), dt.float32,
                           kind="ExternalInput")
    y_d = nc.dram_tensor("y", (CLS, n_batches), dt.float32,
                         kind="ExternalOutput")

    assert n_batches == 128
    GROUPS = globals().get("_GROUPS_OVERRIDE") or [4, 4] + [8] * 15
    S2_CUTS = [64, 96, 120, 128]
    GD = 16 - G_POOL   # G rows on DVE

    with tile.TileContext(nc) as tc:
        with (
            tc.tile_pool(name="const", bufs=1) as cpool,
            tc.tile_pool(name="x", bufs=3) as xpool,
            tc.tile_pool(name="q", bufs=3) as qpool,
            tc.tile_pool(name="hq", bufs=1) as hqpool,
            tc.tile_pool(name="v", bufs=3) as vpool,
            tc.tile_pool(name="oh", bufs=globals().get("_OHBUFS") or 2) as ohpool,
            tc.tile_pool(name="cnt", bufs=1) as cntpool,
            tc.tile_pool(name="ps1", bufs=3, space="PSUM") as ps1pool,
            tc.tile_pool(name="ps2", bufs=1, space="PSUM") as ps2pool,
        ):
            consts = cpool.tile([128, len(_CONSTS)], dt.float32)
            nc.sync.dma_start(
                consts[:], cst_d.ap().broadcast_to((128, len(_CONSTS))))
            cb = {v: consts[:, i:i + 1] for i, v in enumerate(_CONSTS)}

            # prefetch the first groups' x before the big const tiles so the
            # Act chain starts as early as possible
            pre_n = 3
            pre_xg = []
            pb = 0
            for gs in GROUPS[:pre_n]:
                xg = xpool.tile([128, GRP, 3, PJ], dt.float32,
                                tag="xg", name="xg")[:, 0:gs]
                nc.sync.dma_start(xg[:], x_d[:, pb:pb + gs])
                pre_xg.append(xg)
                pb += gs

            w2 = cpool.tile([32, 2, 16, CLS], dt.bfloat16)
            nc.sync.dma_start(w2[:], w2_d[:])
            bias = cpool.tile([CLS, 1], dt.float32)
            nc.sync.dma_start(bias[:], bias_d[:])

            cnt = cntpool.tile([32, n_batches, 16], dt.bfloat16)
            ps2 = ps2pool.tile([CLS, n_batches], dt.float32)

            b0 = 0
            pending = []   # deferred (ps1, pos) cnt copies from prev group
            copied = [0]
            s2_done = [0]

            def flush_pending():
                for ps1t, pos, nb_ in pending:
                    nc.scalar.copy(cnt[:, pos:pos + nb_, :],
                                   ps1t[:, 0:nb_])
                    copied[0] = pos + nb_
                pending.clear()
                for cut in S2_CUTS:
                    if s2_done[0] < cut <= copied[0]:
                        c0 = s2_done[0]
                        for hl in range(2):
                            for gg in range(16):
                                nc.tensor.matmul(ps2[:, c0:cut],
                                                 w2[:, hl, gg],
                                                 cnt[:, c0:cut, gg],
                                                 start=(hl == 0 and gg == 0),
                                                 stop=(hl == 1 and gg == 15))
                        out_t = out[0]
                        if out_t is None:
                            out_t = cpool.tile([CLS, n_batches], dt.float32,
                                               tag="out", name="out")
                            out[0] = out_t
                        nc.vector.tensor_scalar(out_t[:, c0:cut],
                                                ps2[:, c0:cut], 1.0 / N,
                                                bias[:], op.mult, op.add)
                        nc.sync.dma_start(y_d[:, c0:cut], out_t[:, c0:cut])
                        s2_done[0] = cut

            out = [None]
            for gi_, gs in enumerate(GROUPS):
                if gi_ < pre_n:
                    xg = pre_xg[gi_]
                else:
                    xg = xpool.tile([128, GRP, 3, PJ], dt.float32,
                                    tag="xg", name="xg")[:, 0:gs]
                    nc.sync.dma_start(xg[:], x_d[:, b0:b0 + gs])

                s = vpool.tile([128, GRP, PJ], dt.bfloat16, tag="s", name="s")[:, 0:gs]
                nc.scalar.activation(s[:], xg[:, :, 1], AF.Sign)
                s32 = vpool.tile([128, GRP, PJ], dt.bfloat16,
                                 tag="s32", name="s32")[:, 0:gs]
                nc.scalar.mul(s32[:], s[:], -32.0)

                g = vpool.tile([128, GRP, PJ], dt.bfloat16, tag="g", name="g")[:, 0:gs]
                lin = vpool.tile([128, GRP, PJ], dt.bfloat16,
                                 tag="lin", name="lin")[:, 0:gs]
                h = vpool.tile([128, GRP, PJ], dt.bfloat16, tag="h", name="h")[:, 0:gs]
                if gi_ < 2:
                    # head groups: short 3-op joint Act chain + 1x STT
                    # combines so DVE starts as early as possible
                    tj = hqpool.tile([128, 4, 3, PJ], dt.float32,
                                     tag="tj", name="tj")[:, 0:gs]
                    nc.scalar.activation(tj[:], xg[:], AF.Identity,
                                         bias=cb[4.5], scale=2.0)
                    qaj = hqpool.tile([128, 4, 3, PJ], dt.float32,
                                      tag="qaj", name="qaj")[:, 0:gs]
                    nc.scalar.activation(qaj[:], tj[:], AF.Identity,
                                         bias=cb[M23])
                    qj = hqpool.tile([128, 4, 3, PJ], dt.bfloat16,
                                     tag="qj", name="qj")[:, 0:gs]
                    nc.scalar.activation(qj[:], qaj[:], AF.Identity,
                                         bias=cb[-M23])
                    flush_pending()
                    nc.vector.scalar_tensor_tensor(g[:], qj[:, :, 0], 4.0,
                                                   s[:], op.mult, op.add)
                    nc.vector.scalar_tensor_tensor(lin[:], qj[:, :, 1], 16.0,
                                                   qj[:, :, 2], op.mult,
                                                   op.add)
                    nc.vector.scalar_tensor_tensor(h[:], s[:], -32.0, lin[:],
                                                   op.mult, op.add)
                else:
                    qt = []
                    for ci, (scale, coff, cp, cn) in enumerate([
                            (8.0, 18.0, M25, -M25),
                            (32.0, 72.0, M27, -M27),
                            (2.0, 4.5, M23, -M23)]):
                        t = qpool.tile([128, GRP, PJ], dt.float32,
                                       tag=f"t{ci}", name=f"t{ci}")[:, 0:gs]
                        nc.scalar.activation(t[:], xg[:, :, ci], AF.Identity,
                                             bias=cb[coff], scale=scale)
                        qa = qpool.tile([128, GRP, PJ], dt.float32,
                                        tag=f"qa{ci}", name=f"qa{ci}")[:, 0:gs]
                        nc.scalar.activation(qa[:], t[:], AF.Identity,
                                             bias=cb[cp])
                        qq = qpool.tile([128, GRP, PJ], dt.bfloat16,
                                        tag=f"q{ci}", name=f"q{ci}")[:, 0:gs]
                        nc.scalar.activation(qq[:], qa[:], AF.Identity,
                                             bias=cb[cn])
                        qt.append(qq)

                    flush_pending()

                    nc.vector.tensor_tensor(g[:], qt[0][:], s[:], op.add)
                    nc.vector.tensor_tensor(lin[:], qt[1][:], qt[2][:],
                                            op.add)
                    nc.vector.tensor_tensor(h[:], lin[:], s32[:], op.add)

                G = ohpool.tile([128, GRP, 16, PJ], dt.bfloat16,
                                tag="G", name="G")[:, 0:gs]
                H = ohpool.tile([128, GRP, 32, PJ], dt.bfloat16,
                                tag="H", name="H")[:, 0:gs]
                halves = ([(0, gs)] if gi_ < len(GROUPS) - 1 else
                          [(0, gs // 2), (gs // 2, gs)])
                for lo, hi in halves:
                    for m in range(GD, 16):
                        nc.gpsimd.tensor_scalar(G[:, lo:hi, m], g[:, lo:hi],
                                                float(_G_VALS[m]), None,
                                                op.is_equal)
                    for m in range(GD):
                        nc.vector.tensor_scalar(G[:, lo:hi, m], g[:, lo:hi],
                                                float(_G_VALS[m]), None,
                                                op.is_equal)
                    for m in range(32):
                        nc.vector.tensor_scalar(H[:, lo:hi, m], h[:, lo:hi],
                                                float(_H_VALS[m]), None,
                                                op.is_equal)

                    for bb in range(lo, hi, 8):
                        nb_ = min(8, hi - bb)
                        ps1 = ps1pool.tile([32, 8, 16], dt.float32, tag="ps1",
                                           name="ps1")
                        for u in range(nb_):
                            for j in range(PJ):
                                nc.tensor.matmul(ps1[:, u],
                                                 H[:, bb + u, :, j],
                                                 G[:, bb + u, :, j],
                                                 start=(j == 0),
                                                 stop=(j == PJ - 1))
                        pending.append((ps1, b0 + bb, nb_))

                b0 += gs
                if gi_ == len(GROUPS) - 1:
                    flush_pending()


            flush_pending()

    nc.compile()
    return nc


def _aux_inputs(W, b):
    from ml_dtypes import bfloat16 as bf16
    i0 = _GI // 2                                   # [16]
    i1 = (_HI // 8)[:, None] + 4 * (_GI % 2)[None, :]   # [32, 16]
    i2 = _HI % 8                                    # [32]
    vox = 64 * i0[None, :] + 8 * i1 + i2[:, None]   # [32, 16]
    w2f = np.ascontiguousarray(
        W[:, vox].transpose(1, 2, 0)).astype(np.float32)   # [32, 16, 40]
    w2hi = w2f.astype(bf16)
    w2lo = (w2f - w2hi.astype(np.float32)).astype(bf16)
    w2 = np.ascontiguousarray(np.stack([w2hi, w2lo], axis=1))  # [32,2,16,40]
    bias = np.asarray(b, dtype=np.float32).reshape(CLS, 1)
    cst = np.asarray(_CONSTS, dtype=np.float32).reshape(1, len(_CONSTS))
    return w2, bias, cst


def kernel(x, W, b):
    from concourse.bass_utils import run_bass_kernel_spmd

    x = np.asarray(x, dtype=np.float32)
    W = np.asarray(W, dtype=np.float32)
    b = np.asarray(b, dtype=np.float32)

    if BPC not in _CACHE:
        _CACHE[BPC] = _build(BPC)
    nc = _CACHE[BPC]

    w2, bias, cst = _aux_inputs(W, b)
    # [core, 128part, nb, 3coord, PJ]
    shards = x.reshape(NCORES, BPC, 128, PJ, 3).transpose(0, 2, 1, 4, 3)
    in_maps = [
        {"x": np.ascontiguousarray(shards[i]), "w2": w2,
         "bias": bias, "cst": cst}
        for i in range(NCORES)
    ]
    res = run_bass_kernel_spmd(nc, in_maps, list(range(NCORES)))
    return np.concatenate(
        [np.asarray(res.results[i]["y"]).T for i in range(NCORES)],
        axis=0).astype(np.float32)
